# revision 5
# baseline (speedup 1.0000x reference)
"""Trainium2 Bass kernel for nn_DeConv2d (stacked per-channel 3-layer MLP).

Full computation (reference):
  x: [N=8, IC=128, IH=32, IW=32]; per-channel MLP weights stacked along oC=32.
  For each output channel o: a 3-layer MLP (128->256->256->4, relu between)
  applied to every pixel's IC-dim feature vector. Output re-assembled as a
  2x2 "deconv" upsampling: [8, 32, 64, 64].

Strategy:
  - Shard oC across 8 cores (4 channels each); every core gets full x.
  - Feature-major layout on chip: activations stored [feature, pixel] so every
    layer's matmul is out[hb] = W[:, hb].T @ act with zero transposes:
      * x is naturally [IC=128, pix] per batch element -> rhs.
      * W1[o] is naturally [IC=128, H=256] -> lhsT (two 128-col halves).
      * W2[o] [256, 256] -> two K chunks of [128, 256].
      * W3[o] [256, 4]  -> two K chunks of [128, 4].
  - Pixels processed in chunks of 512 (max fp32 moving free dim).
  - Matmuls run as float32r (1 cycle/row at N>=512 vs 4 for fp32).
  - Bias+relu fused: scalar engine activation(Relu, bias) / vector tensor_scalar.
"""

import numpy as np

N, IC, IH, IW = 8, 128, 32, 32
OC, H, KH, KW = 32, 256, 2, 2
NCORES = 8
CPC = OC // NCORES          # channels per core = 4
NPIX = N * IH * IW          # 8192
CH = 512                    # pixel chunk (free dim per matmul)
NCHUNK = NPIX // CH         # 16
P = 128
KK = KH * KW                # 4

MM_DTYPE = "float32r"       # "float32r" | "float32"

_COMPILED = None


def _build_bass():
    import concourse.mybir as mybir
    from concourse import bacc
    from concourse.tile import TileContext

    f32 = mybir.dt.float32
    mm_dt = getattr(mybir.dt, MM_DTYPE)
    AF = mybir.ActivationFunctionType

    nc = bacc.Bacc()
    xr = nc.dram_tensor("xr", [P, NPIX], mm_dt, kind="ExternalInput")
    w1 = nc.dram_tensor("w1", [CPC, IC, H], mm_dt, kind="ExternalInput")
    w2 = nc.dram_tensor("w2", [CPC, H, H], mm_dt, kind="ExternalInput")
    w3 = nc.dram_tensor("w3", [CPC, H, KK], mm_dt, kind="ExternalInput")
    b1 = nc.dram_tensor("b1", [CPC, H], f32, kind="ExternalInput")
    b2 = nc.dram_tensor("b2", [CPC, H], f32, kind="ExternalInput")
    b3 = nc.dram_tensor("b3", [CPC, KK], f32, kind="ExternalInput")
    out = nc.dram_tensor("out", [CPC, KK, NPIX], f32, kind="ExternalOutput")

    with TileContext(nc) as tc:
        with (
            tc.tile_pool(name="xpool", bufs=1) as xpool,
            tc.tile_pool(name="wpool", bufs=1) as wpool,
            tc.tile_pool(name="bpool", bufs=1) as bpool,
            tc.tile_pool(name="fpool", bufs=4) as fpool,
            tc.tile_pool(name="opool", bufs=4) as opool,
            tc.tile_pool(name="pspool", bufs=4, space="PSUM") as pspool,
            tc.tile_pool(name="ps3pool", bufs=2, space="PSUM") as ps3pool,
        ):
            # ---- resident loads: x + all weights/biases for this core ----
            xt = xpool.tile([P, NPIX], mm_dt, tag="x")
            # split the 4MB load across chunks so compute can start early
            for p in range(NCHUNK):
                nc.sync.dma_start(out=xt[:, p * CH:(p + 1) * CH],
                                  in_=xr[:, p * CH:(p + 1) * CH])

            w1t, w2t, w3t, b1t, b2t, b3t = [], [], [], [], [], []
            for o in range(CPC):
                t1 = wpool.tile([P, H], mm_dt, tag=f"w1_{o}")
                nc.sync.dma_start(out=t1, in_=w1[o])
                w1t.append(t1)
                w2t.append([])
                w3t.append([])
                for kb in range(2):
                    t2 = wpool.tile([P, H], mm_dt, tag=f"w2_{o}_{kb}")
                    nc.sync.dma_start(out=t2, in_=w2[o, kb * P:(kb + 1) * P, :])
                    w2t[o].append(t2)
                    t3 = wpool.tile([P, KK], mm_dt, tag=f"w3_{o}_{kb}")
                    nc.sync.dma_start(out=t3, in_=w3[o, kb * P:(kb + 1) * P, :])
                    w3t[o].append(t3)
                bb1, bb2 = [], []
                for hb in range(2):
                    tb = bpool.tile([P, 1], f32, tag=f"b1_{o}_{hb}")
                    nc.sync.dma_start(
                        out=tb, in_=b1[o, hb * P:(hb + 1) * P].rearrange("(p u) -> p u", u=1))
                    bb1.append(tb)
                    tb = bpool.tile([P, 1], f32, tag=f"b2_{o}_{hb}")
                    nc.sync.dma_start(
                        out=tb, in_=b2[o, hb * P:(hb + 1) * P].rearrange("(p u) -> p u", u=1))
                    bb2.append(tb)
                b1t.append(bb1)
                b2t.append(bb2)
                tb = bpool.tile([KK, 1], f32, tag=f"b3_{o}")
                nc.sync.dma_start(out=tb, in_=b3[o].rearrange("(p u) -> p u", u=1))
                b3t.append(tb)

            # ---- compute ----
            for o in range(CPC):
                for p in range(NCHUNK):
                    xs = xt[:, p * CH:(p + 1) * CH]
                    f1 = []
                    for hb in range(2):
                        ps = pspool.tile([P, CH], f32, tag="mm")
                        nc.tensor.matmul(ps, w1t[o][:, hb * P:(hb + 1) * P],
                                         xs, start=True, stop=True)
                        ft = fpool.tile([P, CH], mm_dt, tag=f"f1_{hb}")
                        if hb == 0:
                            nc.scalar.activation(ft, ps, AF.Relu, bias=b1t[o][hb])
                        else:
                            nc.vector.tensor_scalar(
                                ft, ps, b1t[o][hb], 0.0,
                                op0=mybir.AluOpType.add, op1=mybir.AluOpType.max)
                        f1.append(ft)
                    f2 = []
                    for hb in range(2):
                        ps = pspool.tile([P, CH], f32, tag="mm")
                        for kb in range(2):
                            nc.tensor.matmul(
                                ps, w2t[o][kb][:, hb * P:(hb + 1) * P],
                                f1[kb], start=(kb == 0), stop=(kb == 1))
                        ft = fpool.tile([P, CH], mm_dt, tag=f"f2_{hb}")
                        if hb == 0:
                            nc.scalar.activation(ft, ps, AF.Relu, bias=b2t[o][hb])
                        else:
                            nc.vector.tensor_scalar(
                                ft, ps, b2t[o][hb], 0.0,
                                op0=mybir.AluOpType.add, op1=mybir.AluOpType.max)
                        f2.append(ft)
                    ps3 = ps3pool.tile([KK, CH], f32, tag="mm3")
                    for kb in range(2):
                        nc.tensor.matmul(ps3, w3t[o][kb], f2[kb],
                                         start=(kb == 0), stop=(kb == 1))
                    ot = opool.tile([KK, CH], f32, tag="out")
                    nc.scalar.activation(ot, ps3, AF.Identity, bias=b3t[o])
                    nc.sync.dma_start(out=out[o, :, p * CH:(p + 1) * CH], in_=ot)
    nc.finalize()
    return nc


def _get_compiled():
    global _COMPILED
    if _COMPILED is None:
        _COMPILED = _build_bass()
    return _COMPILED


def kernel(x, W1, b1, W2, b2, W3, b3):
    from concourse.bass_utils import run_bass_kernel_spmd

    x = np.asarray(x, dtype=np.float32)
    # [IC, n*IH*IW] feature-major pixel matrix
    xr = np.ascontiguousarray(
        x.reshape(N, IC, IH * IW).transpose(1, 0, 2).reshape(IC, NPIX))

    in_maps = []
    for c in range(NCORES):
        sl = slice(c * CPC, (c + 1) * CPC)
        in_maps.append({
            "xr": xr,
            "w1": np.ascontiguousarray(np.asarray(W1[sl], dtype=np.float32)),
            "w2": np.ascontiguousarray(np.asarray(W2[sl], dtype=np.float32)),
            "w3": np.ascontiguousarray(np.asarray(W3[sl], dtype=np.float32)),
            "b1": np.ascontiguousarray(np.asarray(b1[sl], dtype=np.float32)),
            "b2": np.ascontiguousarray(np.asarray(b2[sl], dtype=np.float32)),
            "b3": np.ascontiguousarray(np.asarray(b3[sl], dtype=np.float32)),
        })

    nc = _get_compiled()
    res = run_bass_kernel_spmd(nc, in_maps, core_ids=list(range(NCORES)))
    # [oC, kk, npix] -> [oC, kh, kw, n, ih, iw] -> [n, oC, ih, kh, iw, kw]
    all_out = np.concatenate([r["out"] for r in res.results], axis=0)
    y = all_out.reshape(OC, KH, KW, N, IH, IW)
    y = y.transpose(3, 0, 4, 1, 5, 2).reshape(N, OC, KH * IH, KW * IW)
    return np.ascontiguousarray(y)


# revision 8
# speedup vs baseline: 1.0562x; 1.0562x over previous
"""Trainium2 Bass kernel for nn_DeConv2d (stacked per-channel 3-layer MLP).

Reference computation:
  x: [N=8, IC=128, IH=32, IW=32]; per-channel MLP weights stacked along oC=32.
  For each output channel o: a 3-layer MLP (128->256->256->4, relu between)
  applied to every pixel's IC-dim feature vector. Output re-assembled as a
  2x2 "deconv" upsampling: [8, 32, 64, 64].

Strategy:
  - Shard oC across 8 cores (4 channels each); every core gets full x.
  - Feature-major on chip: activations stored [feature, pixel] so every layer
    is out[half] = W[:, half].T @ act with zero transposes (x is naturally
    [IC, pix]; W1/W2/W3 are naturally [K, M]).
  - float32r matmuls: 1 cycle/row at N=512 (4x faster than fp32, ~2e-4 rel err).
  - Superchunks of 1024 pixels: matmuls write 512-wide halves of a 2-bank
    [128, 1024] PSUM tile; bias+relu runs once per wide tile (fused, engine
    fixed cost dominates so fewer/wider elementwise ops win), alternating
    between the Scalar and Vector engines.
  - Weights/biases DMA'd before x; output DMAs ride the idle GpSimd queue.
"""

import numpy as np

N, IC, IH, IW = 8, 128, 32, 32
OC, H, KH, KW = 32, 256, 2, 2
NCORES = 8
CPC = OC // NCORES          # channels per core = 4
NPIX = N * IH * IW          # 8192
CH = 512                    # matmul moving free dim (fp32r max)
SC = 1024                   # superchunk (2 matmul chunks, one wide psum tile)
NSC = NPIX // SC            # 8
P = 128
KK = KH * KW                # 4

MM_DTYPE = "float32r"       # "float32r" | "float32"

_COMPILED = None


def _build_bass():
    import concourse.mybir as mybir
    from concourse import bacc
    from concourse.tile import TileContext

    f32 = mybir.dt.float32
    mm_dt = getattr(mybir.dt, MM_DTYPE)
    AF = mybir.ActivationFunctionType
    ALU = mybir.AluOpType

    nc = bacc.Bacc()
    xr = nc.dram_tensor("xr", [P, NPIX], mm_dt, kind="ExternalInput")
    w1 = nc.dram_tensor("w1", [CPC, IC, H], mm_dt, kind="ExternalInput")
    w2 = nc.dram_tensor("w2", [CPC, H, H], mm_dt, kind="ExternalInput")
    w3 = nc.dram_tensor("w3", [CPC, H, KK], mm_dt, kind="ExternalInput")
    b1 = nc.dram_tensor("b1", [CPC, H], f32, kind="ExternalInput")
    b2 = nc.dram_tensor("b2", [CPC, H], f32, kind="ExternalInput")
    b3 = nc.dram_tensor("b3", [CPC, KK], f32, kind="ExternalInput")
    out = nc.dram_tensor("out", [CPC, KK, NPIX], f32, kind="ExternalOutput")

    with TileContext(nc) as tc:
        with (
            tc.tile_pool(name="xpool", bufs=1) as xpool,
            tc.tile_pool(name="wpool", bufs=1) as wpool,
            tc.tile_pool(name="fpool", bufs=2) as fpool,
            tc.tile_pool(name="opool", bufs=4) as opool,
            tc.tile_pool(name="pspool", bufs=3, space="PSUM") as pspool,
            tc.tile_pool(name="ps3pool", bufs=2, space="PSUM") as ps3pool,
        ):
            xt = xpool.tile([P, NPIX], mm_dt, tag="x")
            w1t, w2t, w3t, b1t, b2t = [], [], [], [], []
            b3t = wpool.tile([KK, CPC], f32, tag="b3")

            def load_channel(o):
                t1 = wpool.tile([P, H], mm_dt, tag=f"w1_{o}")
                nc.sync.dma_start(out=t1, in_=w1[o])
                w1t.append(t1)
                tb = wpool.tile([P, 2], f32, tag=f"b1_{o}")
                nc.sync.dma_start(out=tb, in_=b1[o].rearrange("(hb p) -> p hb", hb=2))
                b1t.append(tb)
                # [256, 256] -> [128, 512] with kb chunks side by side
                t2 = wpool.tile([P, 2 * H], mm_dt, tag=f"w2_{o}")
                nc.sync.dma_start(out=t2.rearrange("p (kb h) -> p kb h", kb=2),
                                  in_=w2[o].rearrange("(kb p) h -> p kb h", kb=2))
                w2t.append(t2)
                tb = wpool.tile([P, 2], f32, tag=f"b2_{o}")
                nc.sync.dma_start(out=tb, in_=b2[o].rearrange("(hb p) -> p hb", hb=2))
                b2t.append(tb)
                t3 = wpool.tile([P, 2 * KK], mm_dt, tag=f"w3_{o}")
                nc.sync.dma_start(out=t3.rearrange("p (kb k) -> p kb k", kb=2),
                                  in_=w3[o].rearrange("(kb p) k -> p kb k", kb=2))
                w3t.append(t3)

            # channel 0 weights + first x superchunk first, so compute starts early
            load_channel(0)
            nc.sync.dma_start(out=b3t, in_=b3[:, :].rearrange("o k -> k o"))
            nc.sync.dma_start(out=xt[:, 0:SC], in_=xr[:, 0:SC])
            for o in range(1, CPC):
                load_channel(o)
            for s in range(1, NSC):
                nc.sync.dma_start(out=xt[:, s * SC:(s + 1) * SC],
                                  in_=xr[:, s * SC:(s + 1) * SC])

            eltw = 0  # round-robin parity for ACT/DVE balancing

            def bias_relu(dst, src, bias_ap):
                nonlocal eltw
                eltw += 1
                if eltw % 2:
                    nc.scalar.activation(dst, src, AF.Relu, bias=bias_ap)
                else:
                    nc.vector.tensor_scalar(dst, src, bias_ap, 0.0,
                                            op0=ALU.add, op1=ALU.max)

            def bias_add(dst, src, bias_ap):
                nonlocal eltw
                eltw += 1
                if eltw % 2:
                    nc.scalar.activation(dst, src, AF.Identity, bias=bias_ap)
                else:
                    nc.vector.tensor_scalar(dst, src, bias_ap, 0.0,
                                            op0=ALU.add, op1=ALU.add)

            for s in range(NSC):
                xs = xt[:, s * SC:(s + 1) * SC]
                for o in range(CPC):
                    # ---- layer 1: f1[hb] = relu(W1[:,hb].T @ x + b1[hb]) ----
                    f1 = []
                    for hb in range(2):
                        ps = pspool.tile([P, SC], f32, tag="mm")
                        for c in range(2):
                            nc.tensor.matmul(
                                ps[:, c * CH:(c + 1) * CH],
                                w1t[o][:, hb * P:(hb + 1) * P],
                                xs[:, c * CH:(c + 1) * CH],
                                start=True, stop=True)
                        ft = fpool.tile([P, SC], mm_dt, tag=f"f1_{hb}")
                        bias_relu(ft, ps, b1t[o][:, hb:hb + 1])
                        f1.append(ft)
                    # ---- layer 2: f2[hb] = relu(sum_kb W2[kb][:,hb].T @ f1[kb] + b2) ----
                    f2 = []
                    for hb in range(2):
                        ps = pspool.tile([P, SC], f32, tag="mm")
                        for c in range(2):
                            for kb in range(2):
                                nc.tensor.matmul(
                                    ps[:, c * CH:(c + 1) * CH],
                                    w2t[o][:, kb * H + hb * P:kb * H + (hb + 1) * P],
                                    f1[kb][:, c * CH:(c + 1) * CH],
                                    start=(kb == 0), stop=(kb == 1))
                        ft = fpool.tile([P, SC], mm_dt, tag=f"f2_{hb}")
                        bias_relu(ft, ps, b2t[o][:, hb:hb + 1])
                        f2.append(ft)
                    # ---- layer 3: out = sum_kb W3[kb].T @ f2[kb] + b3 ----
                    for c in range(2):
                        ps3 = ps3pool.tile([KK, CH], f32, tag="mm3")
                        for kb in range(2):
                            nc.tensor.matmul(
                                ps3, w3t[o][:, kb * KK:(kb + 1) * KK],
                                f2[kb][:, c * CH:(c + 1) * CH],
                                start=(kb == 0), stop=(kb == 1))
                        ot = opool.tile([KK, CH], f32, tag="out")
                        bias_add(ot, ps3, b3t[:, o:o + 1])
                        nc.gpsimd.dma_start(
                            out=out[o, :, (2 * s + c) * CH:(2 * s + c + 1) * CH],
                            in_=ot)
    nc.finalize()
    return nc


def _get_compiled():
    global _COMPILED
    if _COMPILED is None:
        _COMPILED = _build_bass()
    return _COMPILED


def make_in_maps(x, W1, b1, W2, b2, W3, b3):
    x = np.asarray(x, dtype=np.float32)
    # [IC, n*IH*IW] feature-major pixel matrix
    xr = np.ascontiguousarray(
        x.reshape(N, IC, IH * IW).transpose(1, 0, 2).reshape(IC, NPIX))
    in_maps = []
    for c in range(NCORES):
        sl = slice(c * CPC, (c + 1) * CPC)
        in_maps.append({
            "xr": xr,
            "w1": np.ascontiguousarray(np.asarray(W1[sl], dtype=np.float32)),
            "w2": np.ascontiguousarray(np.asarray(W2[sl], dtype=np.float32)),
            "w3": np.ascontiguousarray(np.asarray(W3[sl], dtype=np.float32)),
            "b1": np.ascontiguousarray(np.asarray(b1[sl], dtype=np.float32)),
            "b2": np.ascontiguousarray(np.asarray(b2[sl], dtype=np.float32)),
            "b3": np.ascontiguousarray(np.asarray(b3[sl], dtype=np.float32)),
        })
    return in_maps


def assemble(results):
    # [oC, kk, npix] -> [oC, kh, kw, n, ih, iw] -> [n, oC, ih, kh, iw, kw]
    all_out = np.concatenate([r["out"] for r in results], axis=0)
    y = all_out.reshape(OC, KH, KW, N, IH, IW)
    y = y.transpose(3, 0, 4, 1, 5, 2).reshape(N, OC, KH * IH, KW * IW)
    return np.ascontiguousarray(y)


def kernel(x, W1, b1, W2, b2, W3, b3):
    from concourse.bass_utils import run_bass_kernel_spmd

    in_maps = make_in_maps(x, W1, b1, W2, b2, W3, b3)
    nc = _get_compiled()
    res = run_bass_kernel_spmd(nc, in_maps, core_ids=list(range(NCORES)))
    return assemble(res.results)
